# revision 1
# baseline (speedup 1.0000x reference)
"""ContextAwareAttention TRN2 kernel.

Model (reference):
  q_blc = query^T(B,L,C); visual = first 196 tokens, text = last 316
  two   = cosine-window-3 aggregation of text (anchor = token)
  three = cosine-window-5 aggregation of text (anchor = next token)
  out   = a*MHA(query,K,V;W1) + b*MHA([visual;two],K,V;W2) + g*MHA([visual;three],K,V;W3)

Sharding: data-parallel over batch B=32 across 8 cores (4 batches/core).

Per-core design (all feature-major / "transposed" layouts, prepared on host):
  inputs qT/keyT/valT: (4, 768, 512) f32 (feature-major per batch)
  weights per block m: wqT/wkT/wvT (768 d, 768 o); woT padded-head (128, 8*768)
  - window stage: shifts along the free dim of feature-major text; per-token
    dot products via elementwise mul + ones-matmul partition reduction
  - per head: project qT/kT padded-head (96,512); scores s^T(j,i) via one
    K=96 matmul per j-chunk; exp on ACT (no max subtraction: |scores|<~15);
    denominator via ones(128x128) matmul (broadcast over partitions);
    AV with V as stationary operand -> av^T(96,512); normalize by 1/denom
    during the PSUM->SBUF move
  - z-projection token-major, accumulated over the 3 blocks in SBUF
  All matmuls fp32r (full PE rate at N>=256), fp32 accumulation in PSUM.
"""

import numpy as np

import concourse.bass as bass
import concourse.mybir as mybir
import concourse.tile as tile
from concourse import bacc
from concourse import bass_utils

F32 = mybir.dt.float32
F32R = mybir.dt.float32r
OP = mybir.AluOpType
ACTF = mybir.ActivationFunctionType

L, B, D = 512, 32, 768
NH, HD = 8, 96
NR = 196          # visual tokens
T = L - NR        # 316 text tokens
NCORES = 8
BL = B // NCORES  # batches per core
EPS = 1e-8
SCALE = float(1.0 / np.sqrt(HD))

PADL = 3          # left pad of R/inv tiles
RW = PADL + T + 5


def _mm(nc, out, lhsT, rhs, start, stop):
    nc.tensor.matmul(out, lhsT, rhs, start=start, stop=stop)


def _window_stage(nc, sb, ps, ones, ones_f, xqt, twoqt, threeqt):
    """Build two_q^T and three_q^T (feature-major, with visual prefix) from xqt."""
    # R_s[t] = sum_c text[c,t] * text[c,t+s], s=0..3 (t in [0,T))
    rtiles = []
    for s in range(4):
        rs = sb.tile([128, RW], F32, tag="rtile", bufs=5, name=f"r{s}")
        nc.vector.memset(rs[:], 0.0)
        w = T - s
        we = w + (w & 1)  # fp32r matmul needs even moving dim
        rps = ps.tile([128, T], F32, tag="den", bufs=1, name="rps")
        for cc in range(6):
            prod = sb.tile([128, T], F32R, tag="prod", bufs=2, name="prod")
            nc.vector.tensor_tensor(
                prod[:, :w],
                xqt[:, cc * 512 + NR : cc * 512 + NR + w].bitcast(F32),
                xqt[:, cc * 512 + NR + s : cc * 512 + NR + w + s].bitcast(F32),
                op=OP.mult,
            )
            if we > w:
                nc.vector.tensor_scalar_mul(prod[:, w:we], ones_f[:, : we - w], 0.0)
            _mm(nc, rps[:, :we], ones[:], prod[:, :we], start=(cc == 0), stop=(cc == 5))
        nc.scalar.copy(rs[:, PADL : PADL + we], rps[:, :we])
        rtiles.append(rs)
    r0, r1, r2, r3 = rtiles

    # inv[t] = 1 / max(sqrt(R_0[t]), eps); pads stay finite (1/eps)
    inv = sb.tile([128, RW], F32, tag="rtile", bufs=5)
    nc.vector.memset(inv[:], 0.0)
    nc.scalar.sqrt(inv[:, PADL : PADL + T], r0[:, PADL : PADL + T])
    nc.vector.tensor_scalar_max(inv[:], inv[:], EPS)
    nc.vector.reciprocal(inv[:], inv[:])

    def vw(tl, d):
        return tl[:, PADL + d : PADL + d + T]

    # w3_s[t] = R'[.]*inv[t]*inv[t+s]; w5_u[t] = dot5_u[.]*inv[t+1]*inv[t+u]
    w3spec = {-1: (vw(r1, -1), 0, -1), 0: (vw(r0, 0), 0, 0), 1: (vw(r1, 0), 0, 1)}
    w5spec = {
        -2: (vw(r3, -2), 1, -2),
        -1: (vw(r2, -1), 1, -1),
        0: (vw(r1, 0), 1, 0),
        1: (vw(r0, 1), 1, 1),
        2: (vw(r1, 1), 1, 2),
    }

    def weights(spec, nm):
        out = {}
        for s, (dot, ai, wi) in spec.items():
            tmp = sb.tile([128, T], F32, tag="wtmp", bufs=1, name="wtmp")
            nc.gpsimd.tensor_tensor(tmp[:], dot, vw(inv, ai), op=OP.mult)
            w = sb.tile([128, T], F32, tag="wfin", bufs=5, name=f"{nm}_{s}")
            nc.gpsimd.tensor_tensor(w[:], tmp[:], vw(inv, wi), op=OP.mult)
            out[s] = w
        return out

    # out^T[c, t] = sum_s w_s[t] * text[c, t+s]; visual prefix copied from xqt
    def accumulate(dst, wmap, mul_eng):
        shifts = sorted(wmap)
        for cc in range(6):
            nc.scalar.copy(dst[:, cc * 512 : cc * 512 + NR], xqt[:, cc * 512 : cc * 512 + NR])
            acc = dst[:, cc * 512 + NR : cc * 512 + NR + T]
            s0 = shifts[0]
            nc.vector.tensor_tensor(
                acc, wmap[s0][:], xqt[:, cc * 512 + NR + s0 : cc * 512 + NR + T + s0].bitcast(F32),
                op=OP.mult,
            )
            for s in shifts[1:]:
                w = T - s if (cc == 5 and s > 0) else T
                prod2 = sb.tile([128, T], F32, tag="prod2", bufs=2, name="prod2")
                mul_eng.tensor_tensor(
                    prod2[:, :w], wmap[s][:, :w],
                    xqt[:, cc * 512 + NR + s : cc * 512 + NR + w + s].bitcast(F32),
                    op=OP.mult,
                )
                nc.vector.tensor_tensor(acc[:, :w], acc[:, :w].bitcast(F32), prod2[:, :w], op=OP.add)

    accumulate(twoqt, weights(w3spec, "w3"), nc.vector)
    accumulate(threeqt, weights(w5spec, "w5"), nc.gpsimd)


def build_nc():
    nc = bacc.Bacc("TRN2", target_bir_lowering=False, debug=False)

    qt_d = nc.dram_tensor("qt", (BL, D, L), F32R, kind="ExternalInput").ap()
    kt_d = nc.dram_tensor("kt", (BL, D, L), F32R, kind="ExternalInput").ap()
    vt_d = nc.dram_tensor("vt", (BL, D, L), F32R, kind="ExternalInput").ap()
    wq_d = nc.dram_tensor("wq", (3, D, D), F32R, kind="ExternalInput").ap()
    wk_d = nc.dram_tensor("wk", (3, D, D), F32R, kind="ExternalInput").ap()
    wv_d = nc.dram_tensor("wv", (3, D, D), F32R, kind="ExternalInput").ap()
    wo_d = nc.dram_tensor("wo", (3, 128, NH * D), F32R, kind="ExternalInput").ap()
    out_d = nc.dram_tensor("out", (L, BL, D), F32, kind="ExternalOutput").ap()

    with tile.TileContext(nc) as tc:
        with (
            tc.tile_pool(name="cst", bufs=1) as cst,
            tc.tile_pool(name="sb", bufs=1) as sb,
            tc.tile_pool(name="ps", bufs=1, space="PSUM") as ps,
        ):
            ones_f = cst.tile([128, 128], F32)
            nc.vector.memset(ones_f[:], 1.0)
            ones = cst.tile([128, 128], F32R)
            nc.scalar.copy(ones[:], ones_f[:])

            for b in range(BL):
                # ---- load inputs (feature-major) ----
                xqt = sb.tile([128, 6 * 512], F32R, tag="xqt", bufs=1)
                keyt = sb.tile([128, 6 * 512], F32R, tag="keyt", bufs=1)
                valt = sb.tile([128, 6 * 512], F32R, tag="valt", bufs=1)
                nc.sync.dma_start(
                    xqt[:].rearrange("p (c t) -> p c t", t=512),
                    qt_d[b].rearrange("(c p) t -> p c t", p=128),
                )
                nc.sync.dma_start(
                    keyt[:].rearrange("p (c t) -> p c t", t=512),
                    kt_d[b].rearrange("(c p) t -> p c t", p=128),
                )
                nc.sync.dma_start(
                    valt[:].rearrange("p (c t) -> p c t", t=512),
                    vt_d[b].rearrange("(c p) t -> p c t", p=128),
                )

                # ---- window stage: build two_q^T / three_q^T ----
                twoqt = sb.tile([128, 6 * 512], F32R, tag="twoqt", bufs=1)
                threeqt = sb.tile([128, 6 * 512], F32R, tag="threeqt", bufs=1)
                _window_stage(nc, sb, ps, ones, ones_f, xqt, twoqt, threeqt)

                zacc = sb.tile([128, 4 * D], F32, tag="zacc", bufs=1)

                for m in range(3):
                    xmt = (xqt, twoqt, threeqt)[m]

                    wv_sb = sb.tile([128, 6 * D], F32R, tag="wts", bufs=3, name="wv_sb")
                    wq_sb = sb.tile([128, 6 * D], F32R, tag="wts", bufs=3, name="wq_sb")
                    wk_sb = sb.tile([128, 6 * D], F32R, tag="wts", bufs=3, name="wk_sb")
                    for wsb, wd in ((wv_sb, wv_d), (wq_sb, wq_d), (wk_sb, wk_d)):
                        nc.sync.dma_start(
                            wsb[:].rearrange("p (c o) -> p c o", o=D),
                            wd[m].rearrange("(c p) o -> p c o", p=128),
                        )

                    # ---- V projection (token-major, 4 token chunks) ----
                    vsb = sb.tile([128, 4 * D], F32R, tag="vsb", bufs=1)
                    for tk in range(4):
                        vp1 = ps.tile([128, 512], F32, tag="bigA", bufs=1, name="vp1")
                        vp2 = ps.tile([128, 256], F32, tag="bigB", bufs=1, name="vp2")
                        for vp, o0, ow in ((vp1, 0, 512), (vp2, 512, 256)):
                            for dd in range(6):
                                _mm(
                                    nc, vp[:],
                                    valt[:, dd * 512 + tk * 128 : dd * 512 + tk * 128 + 128],
                                    wv_sb[:, dd * D + o0 : dd * D + o0 + ow],
                                    start=(dd == 0), stop=(dd == 5),
                                )
                            nc.scalar.copy(vsb[:, tk * D + o0 : tk * D + o0 + ow], vp[:])

                    # ---- per-head attention ----
                    avp = sb.tile([128, NH * 512], F32R, tag="avp", bufs=1)
                    for h in range(8):
                        qp = ps.tile([96, 512], F32, tag="proj", bufs=2, name="qp")
                        kp = ps.tile([96, 512], F32, tag="proj", bufs=2, name="kp")
                        for dd in range(6):
                            _mm(
                                nc, qp[:],
                                wq_sb[:, dd * D + h * HD : dd * D + h * HD + HD],
                                xmt[:, dd * 512 : (dd + 1) * 512],
                                start=(dd == 0), stop=(dd == 5),
                            )
                        for dd in range(6):
                            _mm(
                                nc, kp[:],
                                wk_sb[:, dd * D + h * HD : dd * D + h * HD + HD],
                                keyt[:, dd * 512 : (dd + 1) * 512],
                                start=(dd == 0), stop=(dd == 5),
                            )
                        qt_h = sb.tile([96, 512], F32R, tag="qtkt", bufs=3, name="qt_h")
                        kt_h = sb.tile([96, 512], F32R, tag="qtkt", bufs=3, name="kt_h")
                        nc.scalar.copy(qt_h[:], qp[:])
                        nc.scalar.copy(kt_h[:], kp[:])

                        den = ps.tile([128, 512], F32, tag="den", bufs=1, name="den")
                        av = ps.tile([96, 512], F32, tag="av", bufs=1, name="av")
                        for jj in range(4):
                            st = ps.tile([128, 512], F32, tag="scores", bufs=2, name="st")
                            _mm(nc, st[:], kt_h[:, jj * 128 : (jj + 1) * 128], qt_h[:],
                                start=True, stop=True)
                            ex = sb.tile([128, 512], F32R, tag="exp", bufs=4, name="ex")
                            nc.scalar.activation(ex[:], st[:], ACTF.Exp, scale=SCALE)
                            _mm(nc, den[:], ones[:], ex[:], start=(jj == 0), stop=(jj == 3))
                            _mm(nc, av[:], vsb[:, jj * D + h * HD : jj * D + h * HD + HD],
                                ex[:], start=(jj == 0), stop=(jj == 3))
                        rbc = sb.tile([128, 512], F32, tag="rbc", bufs=1, name="rbc")
                        nc.vector.reciprocal(rbc[:], den[:])
                        nc.vector.tensor_tensor(
                            avp[:96, h * 512 : (h + 1) * 512], av[:], rbc[:96, :], op=OP.mult
                        )

                    # ---- z projection (token-major) + accumulate over blocks ----
                    wo_sb1 = sb.tile([128, 4 * D], F32R, tag="wts", bufs=3, name="wo_sb1")
                    wo_sb2 = sb.tile([128, 4 * D], F32R, tag="wts", bufs=3, name="wo_sb2")
                    nc.sync.dma_start(wo_sb1[:], wo_d[m, :, : 4 * D])
                    nc.sync.dma_start(wo_sb2[:], wo_d[m, :, 4 * D :])
                    for tk in range(4):
                        zp1 = ps.tile([128, 512], F32, tag="bigA", bufs=1, name="zp1")
                        zp2 = ps.tile([128, 256], F32, tag="bigB", bufs=1, name="zp2")
                        for zp, o0, ow in ((zp1, 0, 512), (zp2, 512, 256)):
                            for h in range(8):
                                wos = wo_sb1 if h < 4 else wo_sb2
                                _mm(
                                    nc, zp[:],
                                    avp[:96, h * 512 + tk * 128 : h * 512 + tk * 128 + 128],
                                    wos[:96, (h % 4) * D + o0 : (h % 4) * D + o0 + ow],
                                    start=(h == 0), stop=(h == 7),
                                )
                            dstz = zacc[:, tk * D + o0 : tk * D + o0 + ow]
                            if m == 0:
                                nc.scalar.copy(dstz, zp[:])
                            else:
                                nc.vector.tensor_tensor(dstz, dstz, zp[:], op=OP.add)
                        if m == 2:
                            nc.sync.dma_start(
                                out_d[tk * 128 : (tk + 1) * 128, b, :],
                                zacc[:, tk * D : (tk + 1) * D],
                            )

    nc.compile()
    return nc


def _host_prep(query, key, value, w_in1, w_out1, w_in2, w_out2, w_in3, w_out3,
               alpha, beta, gamma):
    query = np.asarray(query, np.float32)
    key = np.asarray(key, np.float32)
    value = np.asarray(value, np.float32)
    qT = np.ascontiguousarray(np.transpose(query, (1, 2, 0)))  # (B, D, L)
    kT = np.ascontiguousarray(np.transpose(key, (1, 2, 0)))
    vT = np.ascontiguousarray(np.transpose(value, (1, 2, 0)))

    wq = np.stack([np.ascontiguousarray(np.asarray(w)[:D].T) for w in (w_in1, w_in2, w_in3)])
    wk = np.stack([np.ascontiguousarray(np.asarray(w)[D : 2 * D].T) for w in (w_in1, w_in2, w_in3)])
    wv = np.stack([np.ascontiguousarray(np.asarray(w)[2 * D :].T) for w in (w_in1, w_in2, w_in3)])

    coefs = [np.float32(alpha), np.float32(beta), np.float32(gamma)]
    wo = np.zeros((3, 128, NH * D), np.float32)
    for m, (w, c) in enumerate(zip((w_out1, w_out2, w_out3), coefs)):
        wt = (np.asarray(w, np.float32).T * c).astype(np.float32)  # (C, o)
        wt = wt.reshape(NH, HD, D)  # (h, 96, o)
        wo[m, :HD] = np.transpose(wt, (1, 0, 2)).reshape(HD, NH * D)

    return qT, kT, vT, wq.astype(np.float32), wk.astype(np.float32), wv.astype(np.float32), wo


_NC_CACHE = []


def kernel(**inputs):
    qT, kT, vT, wq, wk, wv, wo = _host_prep(**inputs)

    if not _NC_CACHE:
        _NC_CACHE.append(build_nc())
    nc = _NC_CACHE[0]

    in_maps = []
    for c in range(NCORES):
        sl = slice(c * BL, (c + 1) * BL)
        in_maps.append({
            "qt": qT[sl], "kt": kT[sl], "vt": vT[sl],
            "wq": wq, "wk": wk, "wv": wv, "wo": wo,
        })

    res = bass_utils.run_bass_kernel_spmd(nc, in_maps, core_ids=list(range(NCORES)))
    out = np.concatenate([res.results[c]["out"] for c in range(NCORES)], axis=1)
    return out.astype(np.float32)



# revision 7
# speedup vs baseline: 1.4521x; 1.4521x over previous
"""ContextAwareAttention TRN2 kernel.

Model (reference):
  q_blc = query^T(B,L,C); visual = first 196 tokens, text = last 316
  two   = cosine-window-3 aggregation of text (anchor = token)
  three = cosine-window-5 aggregation of text (anchor = next token)
  out   = a*MHA(query,K,V;W1) + b*MHA([visual;two],K,V;W2) + g*MHA([visual;three],K,V;W3)

Sharding: data-parallel over batch B=32 across 8 cores (4 batches/core).

Per-core design (feature-major layouts, prepared on host, bf16 activations
and weights with fp32 PSUM accumulation):
  inputs qT/keyT/valT: (4, 128, 3072) bf16 (feature-chunk-major per batch)
  weights resident in SBUF, loaded once per core (not per batch)
  - window stage: shifts along the free dim of feature-major text; per-token
    dot products via elementwise mul + ones-matmul partition reduction
  - V projected into a 97-column-per-head layout whose 97th column is ones,
    so the AV matmul yields the softmax denominator as row 96 for free
    (no separate ones-matmul per score chunk); reciprocal_approx_fast on the
    (1,512) denominator row, gpsimd partition_broadcast to 96 rows
  - per head: project qT/kT (96,512); scores s^T(j,i) via one K=96 matmul
    per j-chunk; exp on ACT (no max subtraction: |scores*scale| small)
  - z-projection token-major, accumulated over the 3 blocks in SBUF (f32)
"""

import numpy as np
import ml_dtypes

import concourse.bass as bass
import concourse.mybir as mybir
import concourse.tile as tile
from concourse import bacc
from concourse import bass_utils

F32 = mybir.dt.float32
BF16 = mybir.dt.bfloat16
NPBF16 = ml_dtypes.bfloat16
OP = mybir.AluOpType
ACTF = mybir.ActivationFunctionType

L, B, D = 512, 32, 768
NH, HD = 8, 96
NR = 196          # visual tokens
T = L - NR        # 316 text tokens
NCORES = 8
BL = B // NCORES  # batches per core
EPS = 1e-8
SCALE = float(1.0 / np.sqrt(HD))

PADL = 3          # left pad of R/inv tiles
RW = PADL + T + 5
HD1 = HD + 1      # 97: head dims + ones column for denominator


def _mm(nc, out, lhsT, rhs, start, stop):
    nc.tensor.matmul(out, lhsT, rhs, start=start, stop=stop)


def _window_stage(nc, sb, ps, ones, xqt, twoqt, threeqt):
    """Build two_q^T and three_q^T (feature-major, with visual prefix) from xqt."""
    # R_s[t] = sum_c text[c,t] * text[c,t+s], s=0..3 (t in [0,T))
    rtiles = []
    for s in range(4):
        rs = sb.tile([128, RW], F32, tag="rtile", bufs=5, name=f"r{s}")
        nc.vector.memset(rs[:], 0.0)
        w = T - s
        we = w + (w & 1)  # matmul wants even moving dim
        rps = ps.tile([128, 512], F32, tag="vz", bufs=2, name="rps")
        for cc in range(6):
            prod = sb.tile([128, T], BF16, tag="prod", bufs=2, name="prod")
            nc.vector.tensor_tensor(
                prod[:, :w],
                xqt[:, cc * 512 + NR : cc * 512 + NR + w],
                xqt[:, cc * 512 + NR + s : cc * 512 + NR + w + s],
                op=OP.mult,
            )
            if we > w:
                nc.vector.memset(prod[:, w:we], 0.0)
            _mm(nc, rps[:, :we], ones[:], prod[:, :we], start=(cc == 0), stop=(cc == 5))
        nc.scalar.copy(rs[:, PADL : PADL + we], rps[:, :we])
        rtiles.append(rs)
    r0, r1, r2, r3 = rtiles

    # inv[t] = 1 / max(sqrt(R_0[t]), eps); pads stay finite (1/eps)
    inv = sb.tile([128, RW], F32, tag="rtile", bufs=5)
    nc.vector.memset(inv[:], 0.0)
    nc.scalar.sqrt(inv[:, PADL : PADL + T], r0[:, PADL : PADL + T])
    nc.vector.tensor_scalar_max(inv[:], inv[:], EPS)
    nc.vector.reciprocal(inv[:], inv[:])

    def vw(tl, d):
        return tl[:, PADL + d : PADL + d + T]

    # w3_s[t] = R'[.]*inv[t]*inv[t+s]; w5_u[t] = dot5_u[.]*inv[t+1]*inv[t+u]
    w3spec = {-1: (vw(r1, -1), 0, -1), 0: (vw(r0, 0), 0, 0), 1: (vw(r1, 0), 0, 1)}
    w5spec = {
        -2: (vw(r3, -2), 1, -2),
        -1: (vw(r2, -1), 1, -1),
        0: (vw(r1, 0), 1, 0),
        1: (vw(r0, 1), 1, 1),
        2: (vw(r1, 1), 1, 2),
    }

    def weights(spec, nm):
        out = {}
        for s, (dot, ai, wi) in spec.items():
            tmp = sb.tile([128, T], F32, tag="wtmp", bufs=1, name="wtmp")
            nc.gpsimd.tensor_tensor(tmp[:], dot, vw(inv, ai), op=OP.mult)
            w = sb.tile([128, T], BF16, tag="wfin", bufs=5, name=f"{nm}_{s}")
            nc.gpsimd.tensor_tensor(w[:], tmp[:], vw(inv, wi), op=OP.mult)
            out[s] = w
        return out

    # out^T[c, t] = sum_s w_s[t] * text[c, t+s]; visual prefix copied from xqt
    def accumulate(dst, wmap, mul_eng):
        shifts = sorted(wmap)
        for cc in range(6):
            nc.scalar.copy(dst[:, cc * 512 : cc * 512 + NR], xqt[:, cc * 512 : cc * 512 + NR])
            acc = dst[:, cc * 512 + NR : cc * 512 + NR + T]
            s0 = shifts[0]
            nc.vector.tensor_tensor(
                acc, wmap[s0][:], xqt[:, cc * 512 + NR + s0 : cc * 512 + NR + T + s0],
                op=OP.mult,
            )
            for s in shifts[1:]:
                w = T - s if (cc == 5 and s > 0) else T
                prod2 = sb.tile([128, T], BF16, tag="prod2", bufs=2, name="prod2")
                mul_eng.tensor_tensor(
                    prod2[:, :w], wmap[s][:, :w],
                    xqt[:, cc * 512 + NR + s : cc * 512 + NR + w + s],
                    op=OP.mult,
                )
                nc.vector.tensor_tensor(acc[:, :w], acc[:, :w], prod2[:, :w], op=OP.add)

    accumulate(twoqt, weights(w3spec, "w3"), nc.vector)
    accumulate(threeqt, weights(w5spec, "w5"), nc.gpsimd)


def build_nc():
    nc = bacc.Bacc("TRN2", target_bir_lowering=False, debug=False)

    qt_d = nc.dram_tensor("qt", (BL, 128, 6 * 512), BF16, kind="ExternalInput").ap()
    kt_d = nc.dram_tensor("kt", (BL, 128, 6 * 512), BF16, kind="ExternalInput").ap()
    vt_d = nc.dram_tensor("vt", (BL, 128, 6 * 512), BF16, kind="ExternalInput").ap()
    wq_d = nc.dram_tensor("wq", (3, 128, 6 * D), BF16, kind="ExternalInput").ap()
    wk_d = nc.dram_tensor("wk", (3, 128, 6 * D), BF16, kind="ExternalInput").ap()
    wv_d = nc.dram_tensor("wv", (3, 128, 6 * D), BF16, kind="ExternalInput").ap()
    wo_d = nc.dram_tensor("wo", (3, 128, NH * D), BF16, kind="ExternalInput").ap()
    out_d = nc.dram_tensor("out", (L, BL, D), F32, kind="ExternalOutput").ap()

    with tile.TileContext(nc) as tc:
        with (
            tc.tile_pool(name="cst", bufs=1) as cst,
            tc.tile_pool(name="sb", bufs=1) as sb,
            tc.tile_pool(name="ps", bufs=1, space="PSUM") as ps,
        ):
            ones = cst.tile([128, 128], BF16)
            nc.vector.memset(ones[:], 1.0)

            # resident weights, loaded once (after batch-0 xqt DMA below)
            wts = {}
            for m in range(3):
                for nm, wd in (("wq", wq_d), ("wk", wk_d), ("wv", wv_d), ("wo", wo_d)):
                    wts[(nm, m)] = cst.tile(
                        [128, wd.shape[2]], BF16, name=f"{nm}{m}"
                    )

            for b in range(BL):
                # ---- load inputs (feature-major, contiguous) ----
                xqt = sb.tile([128, 6 * 512], BF16, tag="xqt", bufs=2)
                keyt = sb.tile([128, 6 * 512], BF16, tag="keyt", bufs=1)
                valt = sb.tile([128, 6 * 512], BF16, tag="valt", bufs=1)
                nc.sync.dma_start(xqt[:], qt_d[b])
                if b == 0:
                    for (nm, m), wsb in wts.items():
                        wd = {"wq": wq_d, "wk": wk_d, "wv": wv_d, "wo": wo_d}[nm]
                        nc.sync.dma_start(wsb[:], wd[m])
                nc.sync.dma_start(keyt[:], kt_d[b])
                nc.sync.dma_start(valt[:], vt_d[b])

                # ---- window stage: build two_q^T / three_q^T ----
                twoqt = sb.tile([128, 6 * 512], BF16, tag="twoqt", bufs=1)
                threeqt = sb.tile([128, 6 * 512], BF16, tag="threeqt", bufs=1)
                _window_stage(nc, sb, ps, ones, xqt, twoqt, threeqt)

                zacc = sb.tile([128, 4 * D], F32, tag="zacc", bufs=1)

                for m in range(3):
                    xmt = (xqt, twoqt, threeqt)[m]
                    wq_sb, wk_sb = wts[("wq", m)], wts[("wk", m)]
                    wv_sb, wo_sb = wts[("wv", m)], wts[("wo", m)]

                    # ---- V projection into 97-col-per-head layout ----
                    # head slot = [ones | v dims 0..95]; the ones column (d=0)
                    # makes row 0 of the AV matmul the softmax denominator,
                    # landing on partition 0 (engines can't read at a nonzero
                    # partition offset). +1 dummy col so the tk=3 head-6/7
                    # rearrange view stays in range.
                    vsb = sb.tile([128, 4 * NH * HD1 + 1], BF16, tag="vsb", bufs=1)
                    vview = vsb[:, : 4 * NH * HD1].rearrange(
                        "p (t h d) -> p t h d", h=NH, d=HD1
                    )
                    nc.vector.memset(vview[:, :, :, 0:1], 1.0)
                    for tk in range(4):
                        vp1 = ps.tile([128, 512], F32, tag="vz", bufs=2, name="vp1")
                        vp2 = ps.tile([128, 512], F32, tag="vz", bufs=2, name="vp2")
                        for vp, o0, ow in ((vp1, 0, 512), (vp2, 512, 256)):
                            for dd in range(6):
                                _mm(
                                    nc, vp[:, :ow],
                                    valt[:, dd * 512 + tk * 128 : dd * 512 + tk * 128 + 128],
                                    wv_sb[:, dd * D + o0 : dd * D + o0 + ow],
                                    start=(dd == 0), stop=(dd == 5),
                                )
                        base = tk * NH * HD1
                        # heads 0-4 (cols 0:480 of vp1), strided dst (5 heads x 96)
                        d1 = vsb[:, base + 1 : base + 1 + 5 * HD1].rearrange(
                            "p (h d) -> p h d", d=HD1
                        )[:, :, :HD]
                        s1 = vp1[:, :480].rearrange("p (h d) -> p h d", d=HD)
                        nc.scalar.copy(d1, s1)
                        # head 5 split across vp1/vp2
                        nc.vector.tensor_scalar_mul(
                            vsb[:, base + 5 * HD1 + 1 : base + 5 * HD1 + 33], vp1[:, 480:512], 1.0
                        )
                        nc.vector.tensor_scalar_mul(
                            vsb[:, base + 5 * HD1 + 33 : base + 5 * HD1 + 97], vp2[:, 0:64], 1.0
                        )
                        # heads 6-7 (cols 64:256 of vp2)
                        d4 = vsb[:, base + 6 * HD1 + 1 : base + 6 * HD1 + 1 + 2 * HD1].rearrange(
                            "p (h d) -> p h d", d=HD1
                        )[:, :, :HD]
                        s4 = vp2[:, 64:256].rearrange("p (h d) -> p h d", d=HD)
                        nc.scalar.copy(d4, s4)

                    # ---- per-head attention ----
                    # row 0 of avp is a junk row (den * rden ~= 1); the
                    # z-projection contracts K=97 against a zero row 0 of wo
                    avp = sb.tile([HD1, NH * 512], BF16, tag="avp", bufs=1)
                    for h in range(8):
                        qp = ps.tile([96, 512], F32, tag="proj", bufs=2, name="qp")
                        kp = ps.tile([96, 512], F32, tag="proj", bufs=2, name="kp")
                        for dd in range(6):
                            _mm(
                                nc, qp[:],
                                wq_sb[:, dd * D + h * HD : dd * D + h * HD + HD],
                                xmt[:, dd * 512 : (dd + 1) * 512],
                                start=(dd == 0), stop=(dd == 5),
                            )
                        for dd in range(6):
                            _mm(
                                nc, kp[:],
                                wk_sb[:, dd * D + h * HD : dd * D + h * HD + HD],
                                keyt[:, dd * 512 : (dd + 1) * 512],
                                start=(dd == 0), stop=(dd == 5),
                            )
                        qt_h = sb.tile([96, 512], BF16, tag="qtkt", bufs=3, name="qt_h")
                        kt_h = sb.tile([96, 512], BF16, tag="qtkt", bufs=3, name="kt_h")
                        nc.scalar.copy(qt_h[:], qp[:])
                        nc.vector.tensor_scalar_mul(kt_h[:], kp[:], 1.0)

                        av = ps.tile([HD1, 512], F32, tag="av", bufs=2, name="av")
                        for jj in range(4):
                            st = ps.tile([128, 512], F32, tag="st", bufs=2, name="st")
                            _mm(nc, st[:], kt_h[:, jj * 128 : (jj + 1) * 128], qt_h[:],
                                start=True, stop=True)
                            ex = sb.tile([128, 512], BF16, tag="exp", bufs=4, name="ex")
                            nc.scalar.activation(ex[:], st[:], ACTF.Exp, scale=SCALE)
                            _mm(nc, av[:],
                                vsb[:, jj * NH * HD1 + h * HD1 : jj * NH * HD1 + (h + 1) * HD1],
                                ex[:], start=(jj == 0), stop=(jj == 3))
                        # row 0 of av is the softmax denominator
                        rden = sb.tile([1, 512], F32, tag="rden", bufs=2, name="rden")
                        nc.vector.reciprocal_approx_fast(rden[:], av[0:1, :])
                        rbc = sb.tile([HD1, 512], F32, tag="rbc", bufs=2, name="rbc")
                        nc.gpsimd.partition_broadcast(rbc[:], rden[:], channels=HD1)
                        nc.vector.tensor_tensor(
                            avp[:, h * 512 : (h + 1) * 512], av[:, :], rbc[:], op=OP.mult
                        )

                    # ---- z projection (token-major) + accumulate over blocks ----
                    for tk in range(4):
                        zp1 = ps.tile([128, 512], F32, tag="vz", bufs=2, name="zp1")
                        zp2 = ps.tile([128, 512], F32, tag="vz", bufs=2, name="zp2")
                        for zp, o0, ow in ((zp1, 0, 512), (zp2, 512, 256)):
                            for h in range(8):
                                _mm(
                                    nc, zp[:, :ow],
                                    avp[:, h * 512 + tk * 128 : h * 512 + tk * 128 + 128],
                                    wo_sb[:HD1, h * D + o0 : h * D + o0 + ow],
                                    start=(h == 0), stop=(h == 7),
                                )
                            dstz = zacc[:, tk * D + o0 : tk * D + o0 + ow]
                            if m == 0:
                                nc.scalar.copy(dstz, zp[:, :ow])
                            else:
                                nc.vector.tensor_tensor(dstz, dstz, zp[:, :ow], op=OP.add)
                        if m == 2:
                            nc.sync.dma_start(
                                out_d[tk * 128 : (tk + 1) * 128, b, :],
                                zacc[:, tk * D : (tk + 1) * D],
                            )

    nc.compile()
    return nc


def _host_prep(query, key, value, w_in1, w_out1, w_in2, w_out2, w_in3, w_out3,
               alpha, beta, gamma):
    def feat_major(x):
        # (L,B,D) -> (B, 128, 6*512) bf16, [b, p, c*512+t] = x[t, b, c*128+p]
        xT = np.ascontiguousarray(np.transpose(np.asarray(x, np.float32), (1, 2, 0)))
        return np.ascontiguousarray(
            xT.reshape(B, 6, 128, L).transpose(0, 2, 1, 3).reshape(B, 128, 6 * L)
        ).astype(NPBF16)

    qT, kT, vT = feat_major(query), feat_major(key), feat_major(value)

    def w_block(w, lo):
        # (3d,d) slice [lo:lo+d] -> (128, 6*768), [p, c*768+o] = W[o, c*128+p]
        wt = np.asarray(w, np.float32)[lo : lo + D].T  # (in, out)
        return np.ascontiguousarray(
            wt.reshape(6, 128, D).transpose(1, 0, 2).reshape(128, 6 * D)
        )

    wins = (w_in1, w_in2, w_in3)
    wq = np.stack([w_block(w, 0) for w in wins]).astype(NPBF16)
    wk = np.stack([w_block(w, D) for w in wins]).astype(NPBF16)
    wv = np.stack([w_block(w, 2 * D) for w in wins]).astype(NPBF16)

    coefs = [np.float32(alpha), np.float32(beta), np.float32(gamma)]
    wo = np.zeros((3, 128, NH * D), np.float32)
    for m, (w, c) in enumerate(zip((w_out1, w_out2, w_out3), coefs)):
        wt = (np.asarray(w, np.float32).T * c).astype(np.float32)  # (C, o)
        wt = wt.reshape(NH, HD, D)  # (h, 96, o)
        # rows 1..96; row 0 stays zero to cancel avp's junk row in the
        # K=97 z-projection
        wo[m, 1:HD1] = np.transpose(wt, (1, 0, 2)).reshape(HD, NH * D)

    return qT, kT, vT, wq, wk, wv, wo.astype(NPBF16)


_NC_CACHE = []


def kernel(**inputs):
    qT, kT, vT, wq, wk, wv, wo = _host_prep(**inputs)

    if not _NC_CACHE:
        _NC_CACHE.append(build_nc())
    nc = _NC_CACHE[0]

    in_maps = []
    for c in range(NCORES):
        sl = slice(c * BL, (c + 1) * BL)
        in_maps.append({
            "qt": qT[sl], "kt": kT[sl], "vt": vT[sl],
            "wq": wq, "wk": wk, "wv": wv, "wo": wo,
        })

    res = bass_utils.run_bass_kernel_spmd(nc, in_maps, core_ids=list(range(NCORES)))
    out = np.concatenate([res.results[c]["out"] for c in range(NCORES)], axis=1)
    return out.astype(np.float32)
